# revision 21
# baseline (speedup 1.0000x reference)
"""DeepWalk hierarchical-softmax scoring kernel for 8 Trainium2 NeuronCores.

Computation (mirrors the nn.Module reference):
    path = heap ancestors of leaf u_k           (L ~ 19-20 static ints)
    emd  = emd_weight[v_j]                      [128]
    hv   = hs_weight[path]                      [L, 128]
    out  = -prod(log_sigmoid(hv @ emd))         scalar f32

Sharding: feature-parallel (column) sharding of both embedding tables —
core c owns dims [16c, 16c+16) of every row. Each core gathers the same
(v_j, path) rows from its own shard with ONE indirect DMA, computes
partial dots over its 16 dims, and a single small collective
reconstructs the full dots on every core; the log-sigmoid + product
epilogue runs replicated and core 0's scalar is returned. This needs one
tiny collective (row sharding would need two: an emd broadcast AND a
log-prob reduce, since only one core owns row v_j).

The collective is a ReduceScatter over an 8x-replicated partial-dot
vector: every core contributes [pd|pd|...|pd] (8 chunks of L) and chunk
c of the elementwise sum — the full dots — lands on core c. Same result
on every core, and it prices at 1x the collective overhead in the
cost model where AllReduce prices at 1.875x.
"""

import contextlib

import numpy as np

import concourse.bass as bass
import concourse.mybir as mybir
from concourse.bass_utils import run_bass_kernel_spmd

NUM_V = 1_000_000
EMD_DIM = 128
N_CORES = 8
DSH = EMD_DIM // N_CORES  # 16 dims per core
F32 = mybir.dt.float32
I32 = mybir.dt.int32


def hs_path(u_k: int, num_V: int = NUM_V) -> list[int]:
    """Heap indices of all ancestors of leaf u_k, down-to-root (incl. 0)."""
    n = num_V - 1 + u_k
    path = []
    while n > 0:
        n = (n - 1) // 2
        path.append(n)
    return path


def build_module(v_j: int, u_k: int):
    """Build the per-core Bass module. v_j/u_k are compile-time constants,
    mirroring the reference where the path is a static int array."""
    path = hs_path(u_k)
    L = len(path)
    nc = bass.Bass(num_devices=N_CORES)

    emd = nc.dram_tensor("emd", [NUM_V, DSH], F32, kind="ExternalInput")
    hs = nc.dram_tensor("hs", [NUM_V - 1, DSH], F32, kind="ExternalInput")
    idx = nc.dram_tensor("idx", [1, L], I32, kind="ExternalInput")
    out = nc.dram_tensor("out", [1, 1], F32, kind="ExternalOutput")
    cc_in = nc.dram_tensor("cc_in", [1, N_CORES * L], F32)
    cc_out = nc.dram_tensor("cc_out", [1, L], F32)

    ctx = contextlib.ExitStack()
    with ctx:
        idxt = ctx.enter_context(nc.sbuf_tensor("idxt", [L, 1], I32))
        hv = ctx.enter_context(nc.sbuf_tensor("hv", [L, DSH], F32))
        ev = ctx.enter_context(nc.sbuf_tensor("ev", [L, DSH], F32))
        tmp = ctx.enter_context(nc.sbuf_tensor("tmp", [L, DSH], F32))
        pd = ctx.enter_context(nc.sbuf_tensor("pd", [L, 1], F32))
        dots = ctx.enter_context(nc.sbuf_tensor("dots", [1, L], F32))
        ea = ctx.enter_context(nc.sbuf_tensor("ea", [1, L], F32))
        sp = ctx.enter_context(nc.sbuf_tensor("sp", [1, L], F32))
        lt = ctx.enter_context(nc.sbuf_tensor("lt", [1, L], F32))
        lsum = ctx.enter_context(nc.sbuf_tensor("lsum", [1, 1], F32))
        res = ctx.enter_context(nc.sbuf_tensor("res", [1, 1], F32))
        warm = ctx.enter_context(nc.sbuf_tensor("warm", [1, 1], F32))
        dma_sem = ctx.enter_context(nc.semaphore("dma_sem"))
        idx_sem = ctx.enter_context(nc.semaphore("idx_sem"))
        g_sem = ctx.enter_context(nc.semaphore("g_sem"))
        v_sem = ctx.enter_context(nc.semaphore("v_sem"))
        s_sem = ctx.enter_context(nc.semaphore("s_sem"))
        cc_sem = ctx.enter_context(nc.semaphore("cc_sem"))
        block = ctx.enter_context(nc.Block())

        @block.sync
        def _(sync):
            # path row indices -> one per partition (for the indirect gather)
            sync.dma_start(out=idxt[:, :], in_=idx[0:1, :]).then_inc(idx_sem, 16)
            # center embedding row, replicated across the L partitions
            sync.dma_start(
                out=ev[:, :], in_=emd[v_j : v_j + 1, :].broadcast_to([L, DSH])
            ).then_inc(dma_sem, 16)

            # partial dots -> DRAM, replicated 8x: cc_in[r*L + l] = pd[l]
            sync.wait_ge(v_sem, 1)
            with nc.allow_non_contiguous_dma(
                reason="152 x 4B descriptors in one instruction, intentional"
            ):
                sync.dma_start(
                    out=bass.AP(cc_in, 0, [[1, L], [L, N_CORES]]),
                    in_=pd[0:L, 0:1].broadcast_to([L, N_CORES]),
                ).then_inc(dma_sem, 16)

            # reduced dots back to SBUF, free-major
            sync.wait_ge(cc_sem, 1)
            sync.dma_start(out=dots[0:1, :], in_=cc_out[0:1, :]).then_inc(dma_sem, 16)

            # final scalar out
            sync.wait_ge(s_sem, 5)
            sync.dma_start(out=out[:, :], in_=res[:, :]).then_inc(dma_sem, 16)

        @block.gpsimd
        def _(gpsimd):
            # gather all L path rows of this core's hs shard in ONE indirect
            # DMA: partition l reads row idxt[l]
            gpsimd.wait_ge(idx_sem, 16)
            gpsimd.indirect_dma_start(
                out=hv[:, :],
                out_offset=None,
                in_=hs[:, :],
                in_offset=bass.IndirectOffsetOnAxis(ap=idxt[:, :1], axis=0),
            ).then_inc(g_sem, 16)

            # combine partial dots across cores (single collective)
            gpsimd.wait_ge(dma_sem, 32)
            gpsimd.collective_compute(
                "ReduceScatter",
                mybir.AluOpType.add,
                replica_groups=[list(range(N_CORES))],
                ins=[cc_in[:, :]],
                outs=[cc_out[:, :]],
            ).then_inc(cc_sem, 1)

        @block.vector
        def _(vector):
            # pd[l] = sum_d hv[l,d] * ev[l,d]
            vector.wait_ge(dma_sem, 16)
            vector.wait_ge(g_sem, 16)
            vector.scalar_tensor_tensor(
                out=tmp[:, :],
                in0=hv[:, :],
                scalar=1.0,
                in1=ev[:, :],
                op0=mybir.AluOpType.mult,
                op1=mybir.AluOpType.mult,
                accum_out=pd[:, :],
            ).then_inc(v_sem, 1)

        @block.scalar
        def _(scalar):
            # Dummy activation issued before any wait: triggers the ACT
            # table-set load (~2.7us) concurrently with the gather+collective
            # phase instead of on the critical path. Exp and Ln share the
            # `natural_log_exp_and_others` set, so one load covers both.
            scalar.activation(
                warm[:, :],
                nc.const_aps.tensor(0.0, (1, 1)),
                mybir.ActivationFunctionType.Exp,
            ).then_inc(s_sem, 1)

            # sp = softplus(-dots) = log(exp(-dots) + 1) = -log_sigmoid(dots)
            # (this build's ACT tables have no softplus entry; ln+exp live in
            # one table set). prod(sp) = exp(sum(ln(sp))) via the Ln accum.
            scalar.wait_ge(dma_sem, 48)
            scalar.activation(
                ea[:, :],
                dots[:, :],
                mybir.ActivationFunctionType.Exp,
                scale=-1.0,
            ).then_inc(s_sem, 1)
            # ACT pipeline does not forward: same-engine RAW needs waits
            scalar.wait_ge(s_sem, 2)
            scalar.activation(
                sp[:, :],
                ea[:, :],
                mybir.ActivationFunctionType.Ln,
                bias=1.0,
            ).then_inc(s_sem, 1)
            scalar.wait_ge(s_sem, 3)
            scalar.activation(
                lt[:, :],
                sp[:, :],
                mybir.ActivationFunctionType.Ln,
                accum_out=lsum[:, :],
            ).then_inc(s_sem, 1)
            scalar.wait_ge(s_sem, 4)
            scalar.activation(
                res[:, :],
                lsum[:, :],
                mybir.ActivationFunctionType.Exp,
            ).then_inc(s_sem, 1)

    # res = prod(sp) = (-1)^L prod(logsig); answer = -prod(logsig), so for odd
    # L the answer is res itself, for even L it is -res (host applies sign).
    sign = 1.0 if L % 2 == 1 else -1.0
    return nc, L, sign


_cache: dict = {}


def _get_module(v_j: int, u_k: int):
    key = (v_j, u_k)
    if key not in _cache:
        _cache[key] = build_module(v_j, u_k)
    return _cache[key]


def shard_inputs(emd_np: np.ndarray, hs_np: np.ndarray, u_k: int):
    idx_row = np.asarray(hs_path(u_k), dtype=np.int32).reshape(1, -1)
    return [
        {
            "emd": np.ascontiguousarray(emd_np[:, c * DSH : (c + 1) * DSH]),
            "hs": np.ascontiguousarray(hs_np[:, c * DSH : (c + 1) * DSH]),
            "idx": idx_row,
        }
        for c in range(N_CORES)
    ]


def kernel(v_j, u_k, emd_weight, hs_weight) -> np.ndarray:
    v_j = int(v_j)
    u_k = int(u_k)
    emd_np = np.asarray(emd_weight, dtype=np.float32)
    hs_np = np.asarray(hs_weight, dtype=np.float32)
    assert emd_np.shape == (NUM_V, EMD_DIM), emd_np.shape
    assert hs_np.shape == (NUM_V - 1, EMD_DIM), hs_np.shape

    nc, L, sign = _get_module(v_j, u_k)
    in_maps = shard_inputs(emd_np, hs_np, u_k)
    results = run_bass_kernel_spmd(nc, in_maps, list(range(N_CORES))).results
    val = sign * float(results[0]["out"][0, 0])
    return np.float32(val)


# revision 22
# speedup vs baseline: 1.0191x; 1.0191x over previous
"""DeepWalk hierarchical-softmax scoring kernel for 8 Trainium2 NeuronCores.

Computation (mirrors the nn.Module reference):
    path = heap ancestors of leaf u_k           (L ~ 19-20 static ints)
    emd  = emd_weight[v_j]                      [128]
    hv   = hs_weight[path]                      [L, 128]
    out  = -prod(log_sigmoid(hv @ emd))         scalar f32

Sharding: feature-parallel (column) sharding of both embedding tables —
core c owns dims [16c, 16c+16) of every row. Each core gathers the same
(v_j, path) rows from its own shard with ONE indirect DMA, computes
partial dots over its 16 dims, and a single small collective
reconstructs the full dots on every core; the log-sigmoid + product
epilogue runs replicated and core 0's scalar is returned. This needs one
tiny collective (row sharding would need two: an emd broadcast AND a
log-prob reduce, since only one core owns row v_j).

The collective is a ReduceScatter over an 8x-replicated partial-dot
vector: every core contributes [pd|pd|...|pd] (8 chunks of L) and chunk
c of the elementwise sum — the full dots — lands on core c. Same result
on every core, and it prices at 1x the collective overhead in the
cost model where AllReduce prices at 1.875x.
"""

import contextlib

import numpy as np

import concourse.bass as bass
import concourse.mybir as mybir
from concourse.bass_utils import run_bass_kernel_spmd

NUM_V = 1_000_000
EMD_DIM = 128
N_CORES = 8
DSH = EMD_DIM // N_CORES  # 16 dims per core
F32 = mybir.dt.float32
I32 = mybir.dt.int32


def hs_path(u_k: int, num_V: int = NUM_V) -> list[int]:
    """Heap indices of all ancestors of leaf u_k, down-to-root (incl. 0)."""
    n = num_V - 1 + u_k
    path = []
    while n > 0:
        n = (n - 1) // 2
        path.append(n)
    return path


def build_module(v_j: int, u_k: int):
    """Build the per-core Bass module. v_j/u_k are compile-time constants,
    mirroring the reference where the path is a static int array."""
    path = hs_path(u_k)
    L = len(path)
    nc = bass.Bass(num_devices=N_CORES)

    emd = nc.dram_tensor("emd", [NUM_V, DSH], F32, kind="ExternalInput")
    hs = nc.dram_tensor("hs", [NUM_V - 1, DSH], F32, kind="ExternalInput")
    idx = nc.dram_tensor("idx", [L, 1], I32, kind="ExternalInput")
    out = nc.dram_tensor("out", [1, 1], F32, kind="ExternalOutput")
    cc_in = nc.dram_tensor("cc_in", [1, N_CORES * L], F32)
    cc_out = nc.dram_tensor("cc_out", [1, L], F32)

    ctx = contextlib.ExitStack()
    with ctx:
        hv = ctx.enter_context(nc.sbuf_tensor("hv", [L, DSH], F32))
        ev = ctx.enter_context(nc.sbuf_tensor("ev", [L, DSH], F32))
        tmp = ctx.enter_context(nc.sbuf_tensor("tmp", [L, DSH], F32))
        pd = ctx.enter_context(nc.sbuf_tensor("pd", [L, 1], F32))
        dots = ctx.enter_context(nc.sbuf_tensor("dots", [1, L], F32))
        ea = ctx.enter_context(nc.sbuf_tensor("ea", [1, L], F32))
        sp = ctx.enter_context(nc.sbuf_tensor("sp", [1, L], F32))
        lt = ctx.enter_context(nc.sbuf_tensor("lt", [1, L], F32))
        lsum = ctx.enter_context(nc.sbuf_tensor("lsum", [1, 1], F32))
        res = ctx.enter_context(nc.sbuf_tensor("res", [1, 1], F32))
        warm = ctx.enter_context(nc.sbuf_tensor("warm", [1, 1], F32))
        dma_sem = ctx.enter_context(nc.semaphore("dma_sem"))
        g_sem = ctx.enter_context(nc.semaphore("g_sem"))
        v_sem = ctx.enter_context(nc.semaphore("v_sem"))
        s_sem = ctx.enter_context(nc.semaphore("s_sem"))
        cc_sem = ctx.enter_context(nc.semaphore("cc_sem"))
        block = ctx.enter_context(nc.Block())

        @block.sync
        def _(sync):
            # center embedding row, replicated across the L partitions
            sync.dma_start(
                out=ev[:, :], in_=emd[v_j : v_j + 1, :].broadcast_to([L, DSH])
            ).then_inc(dma_sem, 16)

            # partial dots -> DRAM, replicated 8x: cc_in[r*L + l] = pd[l]
            sync.wait_ge(v_sem, 1)
            with nc.allow_non_contiguous_dma(
                reason="152 x 4B descriptors in one instruction, intentional"
            ):
                sync.dma_start(
                    out=bass.AP(cc_in, 0, [[1, L], [L, N_CORES]]),
                    in_=pd[0:L, 0:1].broadcast_to([L, N_CORES]),
                ).then_inc(dma_sem, 16)

            # reduced dots back to SBUF, free-major
            sync.wait_ge(cc_sem, 1)
            sync.dma_start(out=dots[0:1, :], in_=cc_out[0:1, :]).then_inc(dma_sem, 16)

            # final scalar out
            sync.wait_ge(s_sem, 5)
            sync.dma_start(out=out[:, :], in_=res[:, :]).then_inc(dma_sem, 16)

        @block.gpsimd
        def _(gpsimd):
            # gather all L path rows of this core's hs shard in ONE indirect
            # DMA: partition l reads row idx[l] — the DGE reads the index
            # table directly from DRAM (it is staged as a kernel input), so
            # no index-to-SBUF DMA sits on the critical path
            gpsimd.indirect_dma_start(
                out=hv[:, :],
                out_offset=None,
                in_=hs[:, :],
                in_offset=bass.IndirectOffsetOnAxis(ap=idx[:, :1], axis=0),
            ).then_inc(g_sem, 16)

            # combine partial dots across cores (single collective)
            gpsimd.wait_ge(dma_sem, 32)
            gpsimd.collective_compute(
                "ReduceScatter",
                mybir.AluOpType.add,
                replica_groups=[list(range(N_CORES))],
                ins=[cc_in[:, :]],
                outs=[cc_out[:, :]],
            ).then_inc(cc_sem, 1)

        @block.vector
        def _(vector):
            # pd[l] = sum_d hv[l,d] * ev[l,d]
            vector.wait_ge(dma_sem, 16)
            vector.wait_ge(g_sem, 16)
            vector.scalar_tensor_tensor(
                out=tmp[:, :],
                in0=hv[:, :],
                scalar=1.0,
                in1=ev[:, :],
                op0=mybir.AluOpType.mult,
                op1=mybir.AluOpType.mult,
                accum_out=pd[:, :],
            ).then_inc(v_sem, 1)

        @block.scalar
        def _(scalar):
            # Dummy activation issued before any wait: triggers the ACT
            # table-set load (~2.7us) concurrently with the gather+collective
            # phase instead of on the critical path. Exp and Ln share the
            # `natural_log_exp_and_others` set, so one load covers both.
            scalar.activation(
                warm[:, :],
                nc.const_aps.tensor(0.0, (1, 1)),
                mybir.ActivationFunctionType.Exp,
            ).then_inc(s_sem, 1)

            # sp = softplus(-dots) = log(exp(-dots) + 1) = -log_sigmoid(dots)
            # (this build's ACT tables have no softplus entry; ln+exp live in
            # one table set). prod(sp) = exp(sum(ln(sp))) via the Ln accum.
            scalar.wait_ge(dma_sem, 48)
            scalar.activation(
                ea[:, :],
                dots[:, :],
                mybir.ActivationFunctionType.Exp,
                scale=-1.0,
            ).then_inc(s_sem, 1)
            # ACT pipeline does not forward: same-engine RAW needs waits
            scalar.wait_ge(s_sem, 2)
            scalar.activation(
                sp[:, :],
                ea[:, :],
                mybir.ActivationFunctionType.Ln,
                bias=1.0,
            ).then_inc(s_sem, 1)
            scalar.wait_ge(s_sem, 3)
            scalar.activation(
                lt[:, :],
                sp[:, :],
                mybir.ActivationFunctionType.Ln,
                accum_out=lsum[:, :],
            ).then_inc(s_sem, 1)
            scalar.wait_ge(s_sem, 4)
            scalar.activation(
                res[:, :],
                lsum[:, :],
                mybir.ActivationFunctionType.Exp,
            ).then_inc(s_sem, 1)

    # res = prod(sp) = (-1)^L prod(logsig); answer = -prod(logsig), so for odd
    # L the answer is res itself, for even L it is -res (host applies sign).
    sign = 1.0 if L % 2 == 1 else -1.0
    return nc, L, sign


_cache: dict = {}


def _get_module(v_j: int, u_k: int):
    key = (v_j, u_k)
    if key not in _cache:
        _cache[key] = build_module(v_j, u_k)
    return _cache[key]


def shard_inputs(emd_np: np.ndarray, hs_np: np.ndarray, u_k: int):
    idx_row = np.asarray(hs_path(u_k), dtype=np.int32).reshape(-1, 1)
    return [
        {
            "emd": np.ascontiguousarray(emd_np[:, c * DSH : (c + 1) * DSH]),
            "hs": np.ascontiguousarray(hs_np[:, c * DSH : (c + 1) * DSH]),
            "idx": idx_row,
        }
        for c in range(N_CORES)
    ]


def kernel(v_j, u_k, emd_weight, hs_weight) -> np.ndarray:
    v_j = int(v_j)
    u_k = int(u_k)
    emd_np = np.asarray(emd_weight, dtype=np.float32)
    hs_np = np.asarray(hs_weight, dtype=np.float32)
    assert emd_np.shape == (NUM_V, EMD_DIM), emd_np.shape
    assert hs_np.shape == (NUM_V - 1, EMD_DIM), hs_np.shape

    nc, L, sign = _get_module(v_j, u_k)
    in_maps = shard_inputs(emd_np, hs_np, u_k)
    results = run_bass_kernel_spmd(nc, in_maps, list(range(N_CORES))).results
    val = sign * float(results[0]["out"][0, 0])
    return np.float32(val)
